# revision 1
# baseline (speedup 1.0000x reference)
"""Trainium2 Bass kernel for nn_ALALLaDA windowed-MoE routing blend.

Reference math (see reference.py): out = h + ALPHA * delta, where delta is
written only at masked positions a with >=1 unmasked neighbor in the +-r
window, and equals layer_norm_d of
    mean_t ( sum_k w[a,k] * MLP_k(h[t]) )  over unmasked neighbors t != a,
with w = softmax(h Wr + br) taken at the DESTINATION position a.

Distribution: data-parallel over tokens across 8 cores. The global list of
output positions (masked, >=1 valid neighbor) is split into 8 balanced
contiguous groups (may straddle the batch-row boundary); each core gets the
group plus the +-r source halo -> no collectives.

Algorithmic structure (per core, all layout/indexing prep on host):
  - sources: only unmasked tokens matter (validity 0 otherwise),
    host-compacted,
  - outputs: host-compacted masked positions (~256/core -> full 128-tiles),
  - mm1 (token-major): act0 = gelu(h W1 + b1) for all experts at source
    tokens; b1 rides as an extra contraction row of (h|1)(W1|b1),
  - window sum + destination routing weights fold into one host-built
    banded matrix per expert, At_k[t,a] = band[t,a] * w[a,k], applied by
    associativity BEFORE the W2 contraction:
        Y[(k,f),a] = sum_t act0[t,(k,f)] At_k[t,a]      (cheap matmul)
        num[a,:]   = sum_kf Y[(k,f),a] W2[(k,f),:]      (main matmul 2)
    so no [B,S,K,D] tensor and no device-side routing math at all,
  - 1/cnt normalization cancels inside layer_norm (scale invariance, eps
    perturbation ~1e-3 relative on delta), delta = (num-mu)*ALPHA/std,
  - device returns compacted delta rows (fp8); host scatter-adds into h.
Compute dtype fp8 DoubleRow for the matmuls, f32 PSUM accumulation.

Schedule (v2): startup DMAs fanned across all five engine queues so the
first matmul chain is gated only by its own slab; W2 fully prefetched to
SBUF during phases 1-2 (gpsimd queue) so mm2 never waits on HBM; Y psum
evictions alternate vector/scalar (single-engine CAST was the Y rate
limiter); mm2 runs m-tile-major with LN stats read directly from PSUM and
the last tile's epilogue is the only exposed tail.
"""

import sys

import numpy as np
import ml_dtypes

if "/opt/trn_rl_repo" not in sys.path:
    sys.path.insert(0, "/opt/trn_rl_repo")

ALPHA = 0.08
EPS = 1e-5
N_CORES = 8
P = 128

BF16 = ml_dtypes.bfloat16
FP8 = ml_dtypes.float8_e4m3fn
USE_FP8 = True     # fp8 DoubleRow for the two big matmuls
W1_SCALE = 16.0    # keep fp8 weights out of subnormal range
W2_SCALE = 16.0
Y_SCALE = 8.0      # ysb scale (folded: = AM_SCALE, evict is a plain copy)
AM_SCALE = 8.0     # banded-routing matrix scale for fp8 range


def _ceil_div(a, b):
    return (a + b - 1) // b


def _rup(x, m):
    return _ceil_div(x, m) * m


def _host_prep(h_L, mask, Wr, br, W1, b1, W2, b2, r):
    """Shard + compact on host. Returns (in_maps, dims, out indices)."""
    h_L = np.asarray(h_L, np.float32)
    mask = np.asarray(mask)
    B, S, D = h_L.shape
    K, _, Fh = np.asarray(W1).shape
    h_flat = h_L.reshape(B * S, D)

    masked = mask != 0
    um = ~masked
    umf = um.astype(np.int64)

    # neighbor count per position (excluding center), same clipping as ref
    cnt = np.zeros((B, S), np.int64)
    for o in range(-r, r + 1):
        if o == 0:
            continue
        if o > 0:
            cnt[:, : S - o] += umf[:, o:]
        else:
            cnt[:, -o:] += umf[:, :o]

    b1_nz = bool(np.any(np.asarray(b1)))
    b2_nz = bool(np.any(np.asarray(b2)))

    # balanced split of the global output list into 8 contiguous groups
    glob_out = np.nonzero((masked & (cnt > 0)).reshape(-1))[0]
    n_out = len(glob_out)
    base, rem = divmod(n_out, N_CORES)
    sizes = [base + (1 if i < rem else 0) for i in range(N_CORES)]
    bounds = np.concatenate([[0], np.cumsum(sizes)])

    out_idx, src_idx = [], []
    for c in range(N_CORES):
        oidx = glob_out[bounds[c]: bounds[c + 1]]
        out_idx.append(oidx)
        srcs = []
        for b in np.unique(oidx // S) if len(oidx) else []:
            seg = oidx[oidx // S == b] % S
            lo, hi = max(int(seg.min()) - r, 0), min(int(seg.max()) + r + 1, S)
            srcs.append(np.nonzero(um[b, lo:hi])[0] + lo + b * S)
        src_idx.append(np.concatenate(srcs) if srcs
                       else np.zeros(0, np.int64))

    T_pad = max(32, _rup(max(len(i) for i in src_idx), 32))
    A_pad = max(32, _rup(max(len(i) for i in out_idx), 32))

    D_ext = D + (1 if b1_nz else 0)
    wdt = FP8 if USE_FP8 else BF16
    wsc1 = W1_SCALE if USE_FP8 else 1.0
    wsc2 = W2_SCALE if USE_FP8 else 1.0
    w1r = np.transpose(np.asarray(W1, np.float32), (1, 0, 2)).reshape(D, K * Fh)
    if b1_nz:
        w1r = np.concatenate(
            [w1r, np.asarray(b1, np.float32).reshape(1, K * Fh)], 0)
    w1r = np.ascontiguousarray(w1r * wsc1).astype(wdt)
    w2r = np.asarray(W2, np.float32).reshape(K * Fh, D)
    if b2_nz:
        w2r = np.concatenate([w2r, np.asarray(b2, np.float32).reshape(K, D)], 0)
    w2r = np.ascontiguousarray(w2r * wsc2).astype(wdt)
    Wrf = np.asarray(Wr, np.float32)
    brf = np.asarray(br, np.float32)
    cnt_flat = cnt.reshape(-1)

    # ---- device-layout packing (all partition-major / contiguous so every
    # DMA moves >=2KB per partition line; small strided elements measured
    # ~3x slower than peak) ------------------------------------------------
    ND = _ceil_div(D_ext, P)
    ND2 = _ceil_div(D_ext, 2 * P)
    TCH = _ceil_div(T_pad, P)
    ACH = _ceil_div(A_pad, P)
    NQ = D // 512
    NH = NQ // 2
    KF = K * Fh
    NKF = KF // P
    KK = 1024
    NKK = KF // KK

    # mm1 contraction steps (DoubleRow pairs + leftover singles), mirrored
    # by _build
    dsteps = []
    d = 0
    while d < ND:
        if USE_FP8 and d + 1 < ND and min(P, D_ext - d * P) == P \
                and min(P, D_ext - (d + 1) * P) == P:
            dsteps.append((d, 2))
            d += 2
        else:
            dsteps.append((d, 1))
            d += 1
    NDS = len(dsteps)
    # w1 packed per (kk-pass, step): [P, 2, KK] with contiguous 2KB rows
    w1_packed = np.zeros((NKK, NDS, P, 2, KK), w1r.dtype)
    for kk in range(NKK):
        for si, (d0, nsub) in enumerate(dsteps):
            blk = w1r[d0 * P: d0 * P + min(nsub * P, D_ext - d0 * P),
                      kk * KK:(kk + 1) * KK]
            if nsub == 2:
                w1_packed[kk, si, :, 0, :] = blk[:P]
                w1_packed[kk, si, :, 1, :] = blk[P:]
            else:
                w1_packed[kk, si, : blk.shape[0], 0, :] = blk

    # mm2 contraction steps over kf-chunks (bias row-chunk rides separately)
    csteps = []
    c = 0
    while c < NKF:
        if USE_FP8 and c + 1 < NKF:
            csteps.append((c, 2))
            c += 2
        else:
            csteps.append((c, 1))
            c += 1
    NCS = len(csteps)
    w2_packed = np.zeros((NCS, P, 2, D), w2r.dtype)
    for si, (c0, nsub) in enumerate(csteps):
        blk = w2r[c0 * P: (c0 + nsub) * P, :]
        if nsub == 2:
            w2_packed[si, :, 0, :] = blk[:P]
            w2_packed[si, :, 1, :] = blk[P:]
        else:
            w2_packed[si, :, 0, :] = blk

    in_maps = []
    for c in range(N_CORES):
        sidx, oidx = src_idx[c], out_idx[c]
        Tu, Au = len(sidx), len(oidx)
        hT = np.zeros((2 * P * ND2, T_pad), np.float32)
        hT[:D, :Tu] = h_flat[sidx, :].T
        if b1_nz:
            hT[D, :Tu] = 1.0
        # partition-major: [P, ND2, 2, T_pad] -> per-partition line is one
        # contiguous ND2*2*T_pad segment
        hTp = np.ascontiguousarray(
            hT.reshape(ND2, 2, P, T_pad).transpose(2, 0, 1, 3))
        # destination routing softmax on host (0.1% of the flops, exact f32)
        logits = h_flat[oidx, :] @ Wrf.T + brf
        logits -= logits.max(1, keepdims=True)
        ew = np.exp(logits)
        w = ew / ew.sum(1, keepdims=True)            # [Au, K]
        # banded per-expert matrices At_k[t, a] = band * w[a, k];
        # same-row check: cross-row global diffs only pass |diff|<=r at the
        # row seam, which the row-equality term rejects.
        band = (np.abs(sidx[:, None] - oidx[None, :]) <= r) & \
               (sidx[:, None] != oidx[None, :]) & \
               ((sidx[:, None] // S) == (oidx[None, :] // S))
        am = np.zeros((K, TCH * P, A_pad), np.float32)
        am[:, :Tu, :Au] = band[None, :, :] * w.T[:, None, :]
        # partition-major, k-major layout [P, K, TCH, A_pad]: one DMA per
        # expert with contiguous per-partition lines
        amp = np.ascontiguousarray(
            am.reshape(K, TCH, P, A_pad).transpose(2, 0, 1, 3))
        entry = {
            "hT": np.ascontiguousarray(hTp.astype(wdt)),
            "amat": np.ascontiguousarray(
                (amp * AM_SCALE).astype(wdt) if USE_FP8
                else amp.astype(BF16)),
            "w1": w1_packed,
            "w2": w2_packed,
        }
        if b2_nz:
            entry["b2row"] = np.ascontiguousarray(w2r[NKF * P:, :])
            wc = np.zeros((K, A_pad), np.float32)
            wc[:, :Au] = (w * cnt_flat[oidx][:, None]).T
            if USE_FP8:
                wc *= Y_SCALE
            entry["wcnt"] = np.ascontiguousarray(wc.astype(wdt))
        in_maps.append(entry)

    dims = dict(B=B, S=S, D=D, K=K, Fh=Fh, T_pad=T_pad, A_pad=A_pad,
                D_ext=D_ext, b1_nz=b1_nz, b2_nz=b2_nz)
    return in_maps, dims, out_idx


def _build(dims):
    import concourse.tile as tile
    from concourse import bacc, mybir
    from contextlib import ExitStack

    D, K, Fh = dims["D"], dims["K"], dims["Fh"]
    T_pad, A_pad, D_ext = dims["T_pad"], dims["A_pad"], dims["D_ext"]
    b2_nz = dims["b2_nz"]
    KF = K * Fh
    NKF = KF // P            # kf-chunks (32)
    ND = _ceil_div(D_ext, P)   # contraction chunks of mm1
    TCH = _ceil_div(T_pad, P)  # source-token chunks
    ACH = _ceil_div(A_pad, P)  # output-token chunks
    NQ = D // 512            # 512-wide column tiles of num
    KF2 = NKF + (1 if b2_nz else 0)
    AS = _ceil_div(A_pad, 512)  # N-slices for Y (A_pad normally <= 512)

    def tsz(t):
        return min(P, T_pad - t * P)

    def asz(m):
        return min(P, A_pad - m * P)

    def dsz(d):
        return min(P, D_ext - d * P)

    DT16 = mybir.dt.bfloat16
    DTF = mybir.dt.float32
    DTW = mybir.dt.float8e4 if USE_FP8 else DT16
    DR = mybir.MatmulPerfMode.DoubleRow if USE_FP8 else None
    F = mybir.ActivationFunctionType

    ND2 = _ceil_div(D_ext, 2 * P)   # packed d-chunk pairs in the hT param

    # mirror _host_prep's step lists so the packed params line up
    dsteps = []
    d = 0
    while d < ND:
        if USE_FP8 and d + 1 < ND and dsz(d) == P and dsz(d + 1) == P:
            dsteps.append((d, 2))
            d += 2
        else:
            dsteps.append((d, 1))
            d += 1
    NDS = len(dsteps)
    csteps = []
    c = 0
    while c < NKF:
        if USE_FP8 and c + 1 < NKF:
            csteps.append((c, 2))
            c += 2
        else:
            csteps.append((c, 1))
            c += 1
    NCS = len(csteps)
    KK = 1024
    NKK = KF // KK
    NH = NQ // 2

    nc = bacc.Bacc()
    hT_ext = nc.declare_dram_parameter("hT", [P, ND2, 2, T_pad], DTW,
                                       isOutput=False)
    am_ext = nc.declare_dram_parameter("amat", [P, K, TCH, A_pad], DTW,
                                       isOutput=False)
    w1_ext = nc.declare_dram_parameter("w1", [NKK, NDS, P, 2, KK], DTW,
                                       isOutput=False)
    w2_ext = nc.declare_dram_parameter("w2", [NCS, P, 2, D], DTW,
                                       isOutput=False)
    if b2_nz:
        b2_ext = nc.declare_dram_parameter("b2row", [K, D], DTW,
                                           isOutput=False)
        wc_ext = nc.declare_dram_parameter("wcnt", [K, A_pad], DTW,
                                           isOutput=False)
    out_ext = nc.declare_dram_parameter("out", [ACH, NQ, P, 512], DTW,
                                        isOutput=True)

    with tile.TileContext(nc) as tc, ExitStack() as ctx:
        const = ctx.enter_context(tc.tile_pool(name="const", bufs=1))
        # w2p lives at top level so its SBUF is disjoint from w1p's: if the
        # allocator reuses w1p's space, every W2 slab DMA inherits a WAR
        # wait on the last mm1 reads and the whole 8.4MB W2 stream slips to
        # after phase 2 (measured: transfers pinned until ~69us, stalling
        # mm2 and re-throttling the PE)
        w2p = ctx.enter_context(tc.tile_pool(name="w2p", bufs=NCS + 2))

        hsb = const.tile([P, 2 * ND2, T_pad], DTW)
        amsb = const.tile([P, K, TCH, A_pad], DTW)
        epssb = const.tile([P, 1], DTF)
        if b2_nz:
            wcsb = const.tile([K, A_pad], DTW)

        act0 = const.tile([P, TCH, KF], DTW)     # gelu acts, token-major
        ysb = const.tile([P, NKF, A_pad], DTW)   # banded-mixed activations

        # ---- startup DMA choreography ------------------------------------
        # first mm1 chain is gated by hT pair 0 + the kk=0 W1 slabs; only
        # sync/scalar/gpsimd queues can issue DMAs (~0.65us descriptor prep
        # serialized per queue), so: gpsimd takes hT (pair 0 alone first so
        # it lands earliest, the rest as one big descriptor), while the
        # kk=0 W1 slabs alternate sync/scalar below.  All params are packed
        # partition-major on the host so every line is contiguous.
        # hT pair 0 gates the very first matmul: it goes FIRST on the sync
        # queue (lowest-latency path, ~uncontended transfer); the rest of
        # hT on gpsimd
        nc.sync.dma_start(hsb[:, 0:2, :], hT_ext[:, 0, :, :])
        if ND2 > 1:
            nc.gpsimd.dma_start(
                hsb[:, 2: 2 * ND2, :].rearrange(
                    "p (n two) t -> p n two t", two=2),
                hT_ext[:, 1: ND2, :, :])
        nc.vector.memset(epssb[:], EPS)

        # W2 slabs: full-D rows per cstep pair, issued on the sync queue
        # interleaved with the W1 passes (program order paces the 8.4MB so
        # it never starves the W1 stream) and kept resident through mm2.
        w2slabs = {}
        NW2 = NCS + (1 if b2_nz else 0)

        def issue_w2(si):
            s = w2p.tile([P, 2, D], DTW, tag="w2s", name=f"w2s_{si}")
            eng = nc.sync if si % 2 == 0 else nc.gpsimd
            if si < NCS:
                eng.dma_start(s[:], w2_ext[si, :, :, :])
            else:
                eng.dma_start(s[:K, 0, :], b2_ext[:])
            w2slabs[si] = s

        # ---- phases 1+2+3 in one scope, interleaved tail ------------------
        HC = KK // (2 * P)  # kf-chunks per half-pass
        tsteps = []
        t = 0
        while t < TCH:
            if USE_FP8 and t + 1 < TCH and tsz(t) == P and tsz(t + 1) == P:
                tsteps.append((t, 2))
                t += 2
            else:
                tsteps.append((t, 1))
                t += 1
        with tc.tile_pool(name="w1p", bufs=2 * NDS + 4) as w1p, \
             tc.tile_pool(name="epi", bufs=2) as epi, \
             tc.tile_pool(name="small", bufs=3) as small:

            def emit_y(cidx, ps_y):
                k = cidx // (Fh // P)
                for ns in range(AS):
                    n0, n1 = ns * 512, min((ns + 1) * 512, A_pad)
                    py = ps_y.tile([P, min(512, A_pad)], DTF, tag="py",
                                   name=f"py_{cidx}_{ns}")
                    for ti, (t0, nsub) in enumerate(tsteps):
                        if nsub == 2:
                            nc.tensor.matmul(
                                py[:, : n1 - n0],
                                act0[:, t0: t0 + 2,
                                     cidx * P:(cidx + 1) * P],
                                amsb[:, k, t0: t0 + 2, n0:n1],
                                start=(ti == 0),
                                stop=(ti == len(tsteps) - 1),
                                perf_mode=DR)
                        else:
                            pp = tsz(t0)
                            nc.tensor.matmul(
                                py[:, : n1 - n0],
                                act0[:pp, t0, cidx * P:(cidx + 1) * P],
                                amsb[:pp, k, t0, n0:n1],
                                start=(ti == 0),
                                stop=(ti == len(tsteps) - 1))
                    # evict on vector only: scalar is saturated with gelu
                    # evictions and serializes the phase-2 tail otherwise
                    nc.vector.tensor_copy(ysb[:, cidx, n0:n1],
                                          py[:, : n1 - n0])

            def issue_w1(kk):
                # slab DMAs for pass kk; kk=0/1 alternate sync/scalar so
                # the first two passes' gating slabs land two-queues-wide
                # (scalar is free until the first gelu evictions); later
                # passes ride sync, whose w1p ring-reuse WAR waits pace
                # them automatically one pass ahead of consumption
                sl = {}
                for si in range(NDS):
                    s = w1p.tile([P, 2, KK], DTW, tag="w1s",
                                 name=f"w1s_{kk}_{si}")
                    eng = nc.scalar if (kk <= 1 and si % 2 == 1) else nc.sync
                    if kk == 0 and si == 0:
                        # split so the q=0 half gates MM #1 alone
                        eng.dma_start(s[:, :, 0:512],
                                      w1_ext[kk, si, :, :, 0:512])
                        eng.dma_start(s[:, :, 512:],
                                      w1_ext[kk, si, :, :, 512:])
                    else:
                        eng.dma_start(s[:], w1_ext[kk, si, :, :, :])
                    sl[si] = s
                return sl

            slab_sets = {kk: issue_w1(kk) for kk in range(min(2, NKK))}

            with tc.tile_pool(name="ps_y", bufs=2, space="PSUM") as ps_y, \
                 tc.tile_pool(name="ps_1", bufs=4, space="PSUM") as ps_1:
                for kk in range(NKK):
                    slabs = slab_sets.pop(kk)
                    if kk + 2 < NKK:
                        slab_sets[kk + 2] = issue_w1(kk + 2)
                    if kk == 0:
                        # side inputs ride the gpsimd queue (scalar must
                        # stay free for evictions, W1 owns sync and is
                        # bandwidth-critical).  amsb goes one op per
                        # expert: the DMA fabric gives each in-flight op
                        # only a share of the bandwidth, a single big op
                        # crawls.
                        for k in range(K):
                            nc.gpsimd.dma_start(amsb[:, k, :, :],
                                                am_ext[:, k, :, :])
                        if b2_nz:
                            nc.gpsimd.dma_start(wcsb[:], wc_ext[:])
                    for m in range(TCH):
                        mp = tsz(m)
                        for q in range(KK // 512):
                            pt = ps_1.tile([P, 512], DTF, tag="pt",
                                           name=f"pt_{kk}_{m}_{q}")
                            for si, (d0, nsub) in enumerate(dsteps):
                                if nsub == 2:
                                    nc.tensor.matmul(
                                        pt[:mp, :],
                                        hsb[:, d0: d0 + 2,
                                            m * P: m * P + mp],
                                        slabs[si][:, :,
                                                  q * 512:(q + 1) * 512],
                                        start=(si == 0),
                                        stop=(si == len(dsteps) - 1),
                                        perf_mode=DR)
                                else:
                                    nc.tensor.matmul(
                                        pt[:mp, :],
                                        hsb[: dsz(d0), d0,
                                            m * P: m * P + mp],
                                        slabs[si][: dsz(d0), 0,
                                                  q * 512:(q + 1) * 512],
                                        start=(si == 0),
                                        stop=(si == len(dsteps) - 1))
                            nc.scalar.activation(
                                act0[:mp, m, kk * KK + q * 512:
                                     kk * KK + (q + 1) * 512],
                                pt[:mp, :], F.Gelu,
                                scale=(1.0 / W1_SCALE) if USE_FP8 else 1.0)
                    # Y lags by HALF a pass: mix the previous pass's
                    # second-half kf-chunks and this pass's first half
                    # (whose gelu evictions complete mid-pass); the final
                    # half-pass chunk is deferred into the mm2 interleave
                    ylist = []
                    if kk > 0:
                        ylist += list(range((2 * kk - 1) * HC, 2 * kk * HC))
                    ylist += list(range(2 * kk * HC, (2 * kk + 1) * HC))
                    for cidx in ylist:
                        emit_y(cidx, ps_y)
                # trailing Y half-pass right after the last mm1 pass (its
                # gelu evictions land while these first chains stream)
                for cidx in range((2 * NKK - 1) * HC, 2 * NKK * HC):
                    emit_y(cidx, ps_y)

            # ---- phase 3: num = Y^T W2, layernorm, delta out --------------
            # All 512-col accumulators stay in PSUM (ps_y closed above so
            # ps_2 can have all 8 banks): LN stats and normalize read PSUM
            # directly.  Emission order is chosen so no tile-1 wait
            # threshold covers tile-0 epilogue work: tile 0's stats and
            # epilogue are emitted AFTER tile 1's first chain.
            c2steps = csteps + ([(NKF, 1)] if b2_nz else [])
            with tc.tile_pool(name="ps_2", bufs=8, space="PSUM") as ps_2:
                # W2 stream: sync reaches these issues behind its paced W1
                # preps; gpsimd takes the odd slabs for extra in-flight
                # depth, gated on mid-kernel act0 so it cannot start early
                # and starve the W1 stream
                gate = small.tile([1, 1], DTW, tag="gate", name="gate")
                gk = max(NKK - 3, 0)
                nc.gpsimd.tensor_copy(
                    gate[:], act0[0:1, 0, gk * KK: gk * KK + 1])
                for si in range(NW2):
                    issue_w2(si)
                pst = {}
                stats_t = {}
                for m in range(ACH):
                    stats_t[m] = small.tile([P, NQ, 6], DTF, tag="stats",
                                            name=f"stats_{m}")
                    for qq in range(NQ):
                        pst[(m, qq)] = ps_2.tile([P, 512], DTF, tag="ps2",
                                                 name=f"ps2_{m}_{qq}")

                def mm2_step(m, q, si):
                    c0, nsub = c2steps[si]
                    slab = w2slabs[si]
                    pq = pst[(m, q)]
                    mp = asz(m)
                    kw = dict(start=(si == 0),
                              stop=(si == len(c2steps) - 1))
                    if nsub == 2:
                        nc.tensor.matmul(
                            pq[:mp, :],
                            ysb[:, c0: c0 + 2, m * P: m * P + mp],
                            slab[:, :, q * 512:(q + 1) * 512],
                            perf_mode=DR, **kw)
                    elif c0 < NKF:
                        nc.tensor.matmul(
                            pq[:mp, :],
                            ysb[:, c0, m * P: m * P + mp],
                            slab[:, 0, q * 512:(q + 1) * 512], **kw)
                    else:
                        nc.tensor.matmul(
                            pq[:mp, :],
                            wcsb[:, m * P: m * P + mp],
                            slab[: K, 0, q * 512:(q + 1) * 512], **kw)

                def epilogue(m):
                    mp = asz(m)
                    mv = small.tile([P, 2], DTF, tag="mv", name=f"mv_{m}")
                    nc.vector.bn_aggr(mv[:mp], stats_t[m][:mp])
                    std = small.tile([P, 1], DTF, tag="std",
                                     name=f"std_{m}")
                    nc.scalar.activation(std[:mp], mv[:mp, 1:2], F.Sqrt,
                                         bias=epssb[:mp])
                    s2 = small.tile([P, 1], DTF, tag="s2", name=f"s2_{m}")
                    nc.vector.reciprocal(s2[:mp], std[:mp])
                    nc.vector.tensor_scalar_mul(s2[:mp], s2[:mp], ALPHA)
                    # nb = -mu * s2 so scalar-engine blocks can fuse the
                    # shift into activation's scale/bias form
                    nb = small.tile([P, 1], DTF, tag="nb", name=f"nb_{m}")
                    nc.vector.tensor_scalar(
                        nb[:mp], mv[:mp, 0:1], s2[:mp], -1.0,
                        op0=mybir.AluOpType.mult,
                        op1=mybir.AluOpType.mult)
                    scr = epi.tile([P, D], DTW, tag="scr", name=f"scr_{m}")
                    engs = [nc.sync, nc.gpsimd, nc.sync, nc.gpsimd]
                    for qq in range(NQ):
                        c0 = qq * 512
                        # normalize straight from PSUM, alternating engines
                        if qq % 2 == 0:
                            nc.vector.tensor_scalar(
                                scr[:mp, c0: c0 + 512],
                                pst[(m, qq)][:mp, :],
                                mv[:mp, 0:1], s2[:mp],
                                op0=mybir.AluOpType.subtract,
                                op1=mybir.AluOpType.mult)
                        else:
                            nc.scalar.activation(
                                scr[:mp, c0: c0 + 512],
                                pst[(m, qq)][:mp, :],
                                F.Identity, bias=nb[:mp], scale=s2[:mp])
                        engs[qq].dma_start(
                            out_ext[m, qq, :mp, :],
                            scr[:mp, c0: c0 + 512])

                # tile 0: slab-progressive chains; its stats/epilogue are
                # deferred until after tile 1's first chain is emitted so
                # tile 1's data-wait thresholds (engine-counter semantics)
                # never cover tile-0 epilogue work
                for m in range(ACH - 1):
                    for si in range(len(c2steps)):
                        for q in range(NQ):
                            mm2_step(m, q, si)
                mlast = ACH - 1
                mp = asz(mlast)
                for q in range(NQ):
                    for si in range(len(c2steps)):
                        mm2_step(mlast, q, si)
                    if q == 0:
                        for m in range(ACH - 1):
                            for qq in range(NQ):
                                nc.vector.bn_stats(
                                    stats_t[m][: asz(m), qq, :],
                                    pst[(m, qq)][: asz(m), :])
                            epilogue(m)
                    nc.vector.bn_stats(stats_t[mlast][:mp, q, :],
                                       pst[(mlast, q)][:mp, :])
                epilogue(mlast)

    nc.finalize()
    return nc


def run(inputs, trace=False):
    """Build + execute; returns (full_output, BassKernelResults)."""
    from concourse.bass_utils import run_bass_kernel_spmd

    h_L = np.asarray(inputs["h_L"], np.float32)
    in_maps, dims, out_idx = _host_prep(
        h_L, inputs["mask"], inputs["Wr"], inputs["br"],
        inputs["W1"], inputs["b1"], inputs["W2"], inputs["b2"],
        int(inputs["range_r"]))
    nc = _build(dims)
    res = run_bass_kernel_spmd(nc, in_maps, list(range(N_CORES)), trace=trace)
    out = h_L.copy().reshape(-1, dims["D"])
    for c in range(N_CORES):
        oidx = out_idx[c]
        if len(oidx):
            blk = res.results[c]["out"]          # [ACH, NQ, P, 512]
            ach, nq, _, _ = blk.shape
            flat = np.ascontiguousarray(
                blk.transpose(0, 2, 1, 3)).reshape(ach * P, nq * 512)
            out[oidx, :] += flat[: len(oidx), :].astype(np.float32)
    return out.reshape(h_L.shape), res


def kernel(**inputs):
    out, _ = run(inputs, trace=False)
    return out

